# revision 1
# baseline (speedup 1.0000x reference)
"""Trainium2 Bass kernel for nn_ConvPair (pairwise-MLP message passing).

Reference computation (N=1024 atoms, F=8 feats, H=128 hidden, O=3 out):
    hi = x @ W1[:F];  hj = x @ W1[F:]
    h  = tanh(hi[:,None,:] + hj[None,:,:] + b1)        # [N,N,H]
    h  = tanh(h @ W2 + b2)                             # [N,N,H]
    y  = tanh(h @ W3 + b3)                             # [N,N,O]
    out = y.sum(axis=(1,2))                            # [N]

Sharding: outer atom dim i split across 8 cores (128 i per core); the small
weights and the full atom table are replicated. No cross-core reduction.

Per-core device pipeline, all tensors hidden-major [H=128 partitions, ...]:
  tanh1: ACT  tanh(HJ_T + hib_i)        one instr per i, bias = HIB col
  mm1:   PE   W2.T-contract             2 matmuls (N=512 each) -> PSUM
  tanh2: ACT  tanh(psum + b2) -> SBUF   bias = b2 column
  mm3:   PE   8x (h2_chunk.T @ W3pad)   pairs-on-partitions -> PSUM [128,32]
  tanh3: ACT  tanh(psum), accum_out     per-channel accumulator -> ACC[:,i]
  final: PE   ACC.T @ ones  ->  per-i scalars, ACT copy, DMA out.

b1 is folded into hib host-side; b2 via the ACT bias port; b3 is zeros for
this problem (asserted; a numpy fallback handles the hypothetical nonzero
case exactly).

Wait-discipline: walrus's Activation codegen supports only one semaphore
wait per instruction, so all constants arrive in ONE DMA and each engine
"touches" that DMA's semaphore once in a warmup instruction; afterwards the
steady-state loop only ever ping-pongs ACT<->PE (one foreign sem each).
"""

import json

import numpy as np
from contextlib import ExitStack

import bass_rust
import concourse.bass as bass
import concourse.tile as tile
from concourse import mybir
from concourse.bass_utils import run_bass_kernel_spmd

f32 = mybir.dt.float32
Tanh = mybir.ActivationFunctionType.Tanh

N, F, H, O = 1024, 8, 128, 3
NCORES = 8
IPC = N // NCORES  # 128 atoms (i) per core
NJ = N             # full j dimension on every core
MM_N = 512         # fp32 matmul max moving free dim
OPAD = 4           # W3 padded 3 -> 4 cols (aligned psum writes; pad col = 0)


def _layout(ipc, nj):
    """Column offsets of the packed constant block [H, ncols]."""
    hj = 0
    hib = hj + nj
    w2 = hib + ipc
    b2 = w2 + H
    w3 = b2 + 1
    ones = w3 + OPAD
    ncols = ones + 1
    return dict(hj=hj, hib=hib, w2=w2, b2=b2, w3=w3, ones=ones, ncols=ncols)


# TPB instructions have a single 8-byte events field: 2 sync commands max
# (walrus rejects more).  Queue-engine DMA ops handle their own sync.
_MULTIWAIT_OK = {"DMACopy", "TriggeredCopy", "Call", "ISA"}


def _legalize_waits(nc):
    """Hoist excess semaphore waits from datapath instructions onto chained
    NoOps (one wait each) so every instruction fits walrus's sync budget."""
    j = json.loads(bass_rust.module_to_json_string(nc.m))
    counter = [0]

    def fix_list(insts):
        out = []
        for inst in insts:
            si = inst.get("sync_info")
            waits = (si or {}).get("on_wait", [])
            if si and len(waits) > 1 and inst.get("opcode") not in _MULTIWAIT_OK:
                # keep zero waits on the instruction; one NoOp per wait
                for w in waits:
                    counter[0] += 1
                    out.append({
                        "debug": inst.get("debug", 0),
                        "engine": inst["engine"],
                        "ins": [],
                        "outs": [],
                        "name": f"W-hoist-{counter[0]}",
                        "opcode": "NoOp",
                        "sync_info": {"on_update": [], "on_wait": [w]},
                    })
                si["on_wait"] = []
            out.append(inst)
        return out

    def walk(o):
        if isinstance(o, dict):
            if "instructions" in o and isinstance(o["instructions"], list):
                o["instructions"] = fix_list(o["instructions"])
            for v in o.values():
                walk(v)
        elif isinstance(o, list):
            for v in o:
                walk(v)

    walk(j)
    nc.m = bass_rust.module_from_json_string(json.dumps(j))
    return counter[0]


def _build(ipc, nj, reps=1):
    """Build the per-core Bass program (SPMD: same program, per-core data).

    reps > 1 repeats the main i-loop (recomputing identical results) and is
    used only for differential timing; outputs are unchanged."""
    assert nj % MM_N == 0 and nj % H == 0
    nchunk = nj // H  # stage-3 chunks of 128 pairs
    lay = _layout(ipc, nj)

    nc = bass.Bass()
    cparam = nc.declare_dram_parameter("c", [H, lay["ncols"]], f32, isOutput=False)
    yparam = nc.declare_dram_parameter("y", [ipc, 1], f32, isOutput=True)

    with tile.TileContext(nc) as tc:
        with ExitStack() as ctx:
            consts = ctx.enter_context(tc.tile_pool(name="consts", bufs=1))
            h1p = ctx.enter_context(tc.tile_pool(name="h1p", bufs=3))
            h2p = ctx.enter_context(tc.tile_pool(name="h2p", bufs=3))
            scrp = ctx.enter_context(tc.tile_pool(name="scrp", bufs=1))
            accp = ctx.enter_context(tc.tile_pool(name="accp", bufs=1))
            # PSUM budget (8 banks): ps1 double-buffer 2x2 + ps3 2x1 + warm 1 + fin 1
            psA = ctx.enter_context(tc.tile_pool(name="psA", bufs=2, space="PSUM"))
            psB = ctx.enter_context(tc.tile_pool(name="psB", bufs=2, space="PSUM"))
            psW = ctx.enter_context(tc.tile_pool(name="psW", bufs=1, space="PSUM"))
            psF = ctx.enter_context(tc.tile_pool(name="psF", bufs=1, space="PSUM"))

            C = consts.tile([H, lay["ncols"]], f32)
            nc.sync.dma_start(out=C, in_=cparam[:, :])

            HJ = C[:, lay["hj"]:lay["hj"] + nj]
            W2 = C[:, lay["w2"]:lay["w2"] + H]
            B2 = C[:, lay["b2"]:lay["b2"] + 1]
            W3 = C[:, lay["w3"]:lay["w3"] + OPAD]
            ONES = C[:, lay["ones"]:lay["ones"] + 1]

            ACC = accp.tile([H, ipc], f32)          # [j-offset, i] partial sums
            warm = scrp.tile([H, 1], f32, tag="warm")

            # --- warmups: let ACT and PE observe the const-DMA semaphore
            # (and load the tanh table) on single-wait instructions.
            nc.scalar.activation(out=warm, in_=B2, func=Tanh)
            warm_ps = psW.tile([1, 1], f32)
            nc.tensor.matmul(warm_ps, C[:, lay["w2"]:lay["w2"] + 1],
                             C[:, lay["w2"]:lay["w2"] + 1], start=True, stop=True)

            # --- main loop: groups of G atoms; tanh1/tanh3 batched per group
            G = 8 if ipc % 8 == 0 else (4 if ipc % 4 == 0 else 1)
            for g in [g for _ in range(reps) for g in range(ipc // G)]:
                # DVE broadcast-adds HJ + hib_i into a [128, G*nj] block,
                # then ONE big ACT tanh covers the whole group.
                h1 = h1p.tile([H, G, nj], f32)
                for k in range(G):
                    i = g * G + k
                    nc.vector.tensor_scalar_add(
                        h1[:, k, :], HJ,
                        C[:, lay["hib"] + i:lay["hib"] + i + 1],
                    )
                nc.scalar.activation(out=h1[:, :, :], in_=h1[:, :, :], func=Tanh)

                ps3 = psB.tile([H, G, nchunk, OPAD], f32, tag="s3")
                for k in range(G):
                    ps1 = psA.tile([H, nj], f32)
                    for t in range(nj // MM_N):
                        nc.tensor.matmul(
                            ps1[:, t * MM_N:(t + 1) * MM_N],
                            W2,
                            h1[:, k, t * MM_N:(t + 1) * MM_N],
                            start=True, stop=True,
                        )
                    h2 = h2p.tile([H, nj], f32)
                    nc.scalar.activation(out=h2, in_=ps1, func=Tanh, bias=B2)
                    for cch in range(nchunk):
                        nc.tensor.matmul(
                            ps3[:, k, cch, :],
                            h2[:, cch * H:(cch + 1) * H],
                            W3,
                            start=True, stop=True,
                        )
                # one in-place tanh over the whole group's [128, G*32] block,
                # then DVE free-axis reduce into ACC columns
                nc.scalar.activation(out=ps3[:, :, :, :], in_=ps3[:, :, :, :],
                                     func=Tanh)
                nc.vector.tensor_reduce(
                    out=ACC[:, g * G:(g + 1) * G],
                    in_=ps3.rearrange("p g c o -> p g (c o)"),
                    axis=mybir.AxisListType.X,
                    op=mybir.AluOpType.add,
                )

            # --- reduce over the 128 j-offset partitions: out = ACC.T @ ones
            fin = psF.tile([ipc, 1], f32)
            nc.tensor.matmul(fin, ACC, ONES, start=True, stop=True)
            yout = scrp.tile([ipc, 1], f32, tag="yout")
            nc.scalar.copy(yout, fin)
            nc.sync.dma_start(out=yparam[:, :], in_=yout)

    _legalize_waits(nc)
    return nc


_NC_CACHE = {}


def _get_nc(ipc, nj):
    key = (ipc, nj)
    if key not in _NC_CACHE:
        _NC_CACHE[key] = _build(ipc, nj)
    return _NC_CACHE[key]


def _host_prep(x, W1, b1, ipc, nj):
    """Build the per-core packed const blocks. Returns list of [H,ncols] f32."""
    lay = _layout(ipc, nj)
    hi = x @ W1[:F]          # [N, H]
    hj = x @ W1[F:]          # [N, H]
    hib = hi + b1[None, :]   # fold b1
    hj_t = np.ascontiguousarray(hj[:nj].T)    # [H, nj]
    return lay, hib, hj_t


def kernel(x, W1, b1, W2, b2, W3, b3):
    x = np.asarray(x, np.float32)
    W1 = np.asarray(W1, np.float32)
    b1 = np.asarray(b1, np.float32)
    W2 = np.asarray(W2, np.float32)
    b2 = np.asarray(b2, np.float32)
    W3 = np.asarray(W3, np.float32)
    b3 = np.asarray(b3, np.float32)

    if np.any(b3 != 0.0):
        # Never hit for this problem (spec fills b3 with zeros); exact
        # numpy fallback keeps the kernel correct for arbitrary inputs.
        return _numpy_ref(x, W1, b1, W2, b2, W3, b3)

    lay, hib, hj_t = _host_prep(x, W1, b1, IPC, NJ)
    W3pad = np.zeros((H, OPAD), np.float32)
    W3pad[:, :O] = W3

    in_maps = []
    for c in range(NCORES):
        blk = np.empty((H, lay["ncols"]), np.float32)
        blk[:, lay["hj"]:lay["hj"] + NJ] = hj_t
        blk[:, lay["hib"]:lay["hib"] + IPC] = hib[c * IPC:(c + 1) * IPC].T
        blk[:, lay["w2"]:lay["w2"] + H] = W2
        blk[:, lay["b2"]] = b2
        blk[:, lay["w3"]:lay["w3"] + OPAD] = W3pad
        blk[:, lay["ones"]] = 1.0
        in_maps.append({"c": blk})

    nc = _get_nc(IPC, NJ)
    res = run_bass_kernel_spmd(nc, in_maps, list(range(NCORES)))
    out = np.concatenate(
        [res.results[c]["y"].reshape(IPC) for c in range(NCORES)]
    ).astype(np.float32)
    return out


def _numpy_ref(x, W1, b1, W2, b2, W3, b3):
    hi = x @ W1[:F]
    hj = x @ W1[F:]
    out = np.empty((N,), np.float32)
    for i in range(N):
        h = np.tanh(hi[i][None, :] + hj + b1[None, :])
        h = np.tanh(h @ W2 + b2[None, :])
        y = np.tanh(h @ W3 + b3[None, :])
        out[i] = y.sum()
    return out



# revision 5
# speedup vs baseline: 2.5899x; 2.5899x over previous
"""Trainium2 Bass kernel for nn_ConvPair (pairwise-MLP message passing).

Reference computation (N=1024 atoms, F=8 feats, H=128 hidden, O=3 out):
    hi = x @ W1[:F];  hj = x @ W1[F:]
    h  = tanh(hi[:,None,:] + hj[None,:,:] + b1)        # [N,N,H]
    h  = tanh(h @ W2 + b2)                             # [N,N,H]
    y  = tanh(h @ W3 + b3)                             # [N,N,O]
    out = y.sum(axis=(1,2))                            # [N]

Sharding: outer atom dim i split across 8 cores (128 i per core); weights
and the atom table replicated. No cross-core reduction.

Per-core pipeline, hidden-major [H=128 partitions, ...]:
  The ScalarE (ACT) is the bound engine: all ~34M tanh/core go through its
  128-lane 1 elem/cycle/lane pipe. Everything else is arranged to keep ACT
  saturated:
    z1:    DVE  tensor_scalar_add HJ + hib_i (bf16 4x mode), G=16 atom batch
    tanh1: ACT  one 16384-col instruction per batch (amortize ~352cyc fixed)
    mm2:   PE   W2 stationary bf16 (1 cyc/row + FWL), 2-i blocks into a
                [128,2048] PSUM tile (4 banks, ping-pong x2 = 8 banks)
    tanh2: ACT  tanh(psum + b2) -> h2 bf16 SBUF, one 2048-col instr per block
    mm3:   PE   W3pad stationary [128,4] bf16, h2 moving, col-tiled outputs
                (tile_position=(0,32c)) packed into the last psum bank
    tanh3: ACT  tanh in place + accum_out (free-axis sum) -> ACC column
  Final [128, 64] ACC goes back via DMA; the host sums 6 rows per atom.

bf16 on the matmul path is safe: tolerance is 2e-2 of max|out| ~ 999, i.e.
~20 absolute, while bf16 noise accumulates to ~0.5 here.

b1 is folded into hib host-side; b2/b3 ride the ACT bias port (b3 as a
per-partition column matching the col-tiled mm3 layout).

Wait-discipline: walrus supports limited semaphore waits on datapath
instructions; _legalize_waits hoists extras onto NoOps.
"""

import json

import numpy as np
from contextlib import ExitStack

import bass_rust
import concourse.bass as bass
import concourse.tile as tile
from concourse import mybir
from concourse.bass_utils import run_bass_kernel_spmd

f32 = mybir.dt.float32
bf16 = mybir.dt.bfloat16
Tanh = mybir.ActivationFunctionType.Tanh

N, F, H, O = 1024, 8, 128, 3
NCORES = 8
IPC = N // NCORES   # 128 atoms (i) per core
NJ = N              # full j dimension on every core
MM_N = 512          # max moving free dim per matmul (one PSUM bank)
OPAD = 4            # W3 padded 3 -> 4 cols
G = 16              # z1/tanh1 batch (atoms)
BLK = 2             # atoms per mm2/tanh2 block


def _layout(ipc, nj):
    """Column offsets: cb = packed bf16 block, cf = packed f32 block."""
    hj = 0
    w2 = hj + nj
    w3 = w2 + H
    ncols_b = w3 + OPAD
    hib = 0
    b2 = hib + ipc
    b3c = b2 + 1
    ncols_f = b3c + 1
    return dict(hj=hj, hib=hib, w2=w2, w3=w3, ncols_b=ncols_b,
                b2=b2, b3c=b3c, ncols_f=ncols_f)


# TPB instructions have a single 8-byte events field: 2 sync commands max
# (walrus rejects more).  Queue-engine DMA ops handle their own sync.
_MULTIWAIT_OK = {"DMACopy", "TriggeredCopy", "Call", "ISA"}


def _legalize_waits(nc):
    """Hoist excess semaphore waits from datapath instructions onto chained
    NoOps (one wait each) so every instruction fits walrus's sync budget."""
    j = json.loads(bass_rust.module_to_json_string(nc.m))
    counter = [0]

    def fix_list(insts):
        out = []
        for inst in insts:
            si = inst.get("sync_info")
            waits = (si or {}).get("on_wait", [])
            if si and len(waits) > 1 and inst.get("opcode") not in _MULTIWAIT_OK:
                for w in waits:
                    counter[0] += 1
                    out.append({
                        "debug": inst.get("debug", 0),
                        "engine": inst["engine"],
                        "ins": [],
                        "outs": [],
                        "name": f"W-hoist-{counter[0]}",
                        "opcode": "NoOp",
                        "sync_info": {"on_update": [], "on_wait": [w]},
                    })
                si["on_wait"] = []
            out.append(inst)
        return out

    def walk(o):
        if isinstance(o, dict):
            if "instructions" in o and isinstance(o["instructions"], list):
                o["instructions"] = fix_list(o["instructions"])
            for v in o.values():
                walk(v)
        elif isinstance(o, list):
            for v in o:
                walk(v)

    walk(j)
    nc.m = bass_rust.module_from_json_string(json.dumps(j))
    return counter[0]


def _build(ipc, nj, reps=1, legalize=True):
    """Build the per-core Bass program (SPMD: same program, per-core data).

    reps > 1 repeats the main i-loop (recomputing identical results); used
    only for differential timing, outputs unchanged."""
    assert ipc % G == 0 and G % BLK == 0 and nj == 2 * MM_N
    nblocks = ipc // BLK
    lay = _layout(ipc, nj)

    nc = bass.Bass()
    cbparam = nc.declare_dram_parameter("cb", [H, lay["ncols_b"]], bf16,
                                        isOutput=False)
    cfparam = nc.declare_dram_parameter("cf", [H, lay["ncols_f"]], f32,
                                        isOutput=False)
    yparam = nc.declare_dram_parameter("y", [H, nblocks], f32, isOutput=True)

    with tile.TileContext(nc) as tc:
        with ExitStack() as ctx:
            consts = ctx.enter_context(tc.tile_pool(name="consts", bufs=1))
            z1p = ctx.enter_context(tc.tile_pool(name="z1p", bufs=2))
            h2p = ctx.enter_context(tc.tile_pool(name="h2p", bufs=3))
            scrp = ctx.enter_context(tc.tile_pool(name="scrp", bufs=1))
            accp = ctx.enter_context(tc.tile_pool(name="accp", bufs=1))
            psp = ctx.enter_context(tc.tile_pool(name="psp", bufs=2, space="PSUM"))

            CB = consts.tile([H, lay["ncols_b"]], bf16, tag="cb")
            CF = consts.tile([H, lay["ncols_f"]], f32, tag="cf")
            nc.sync.dma_start(out=CB, in_=cbparam[:, :])
            nc.sync.dma_start(out=CF, in_=cfparam[:, :])

            HJ = CB[:, lay["hj"]:lay["hj"] + nj]
            W2 = CB[:, lay["w2"]:lay["w2"] + H]
            W3 = CB[:, lay["w3"]:lay["w3"] + OPAD]
            B2 = CF[:, lay["b2"]:lay["b2"] + 1]
            B3C = CF[:, lay["b3c"]:lay["b3c"] + 1]

            ACC = accp.tile([H, nblocks], f32)
            warm = scrp.tile([H, 1], f32, tag="warm")

            # warmup: load the tanh table (and touch the const-DMA sems on
            # ACT) outside the steady-state loop.
            nc.scalar.activation(out=warm, in_=B2, func=Tanh)

            for rep in range(reps):
                for B in range(ipc // G):
                    z1 = z1p.tile([H, G, nj], bf16)
                    for k in range(G):
                        i = B * G + k
                        nc.vector.tensor_scalar_add(
                            z1[:, k, :], HJ,
                            CF[:, lay["hib"] + i:lay["hib"] + i + 1],
                        )
                    # tanh1 over the whole batch in one instruction
                    nc.scalar.activation(out=z1[:, :, :], in_=z1[:, :, :],
                                         func=Tanh)
                    for blk in range(G // BLK):
                        g = B * (G // BLK) + blk  # global block id
                        P = psp.tile([H, BLK * nj], f32)
                        for m in range(BLK):
                            for h in range(2):
                                c4 = 2 * m + h
                                nc.tensor.matmul(
                                    P[:, c4 * MM_N:(c4 + 1) * MM_N],
                                    W2,
                                    z1[:, BLK * blk + m, h * MM_N:(h + 1) * MM_N],
                                    start=True, stop=True,
                                )
                        h2 = h2p.tile([H, BLK * nj], bf16)
                        nc.scalar.activation(out=h2, in_=P, func=Tanh, bias=B2)
                        for m in range(BLK):
                            for h in range(2):
                                c4 = 2 * m + h
                                nc.tensor.matmul(
                                    P[32 * c4:32 * c4 + OPAD, 3 * MM_N:4 * MM_N],
                                    W3,
                                    h2[:, c4 * MM_N:(c4 + 1) * MM_N],
                                    start=True, stop=True,
                                    tile_position=(0, 32 * c4),
                                )
                        nc.scalar.activation(
                            out=P[:, 3 * MM_N:4 * MM_N],
                            in_=P[:, 3 * MM_N:4 * MM_N],
                            func=Tanh, bias=B3C,
                            accum_out=ACC[:, g:g + 1],
                        )

            nc.sync.dma_start(out=yparam[:, :], in_=ACC)

    if legalize:
        _legalize_waits(nc)
    return nc


_NC_CACHE = {}


def _get_nc(ipc, nj):
    key = (ipc, nj)
    if key not in _NC_CACHE:
        _NC_CACHE[key] = _build(ipc, nj)
    return _NC_CACHE[key]


def _host_prep(x, W1, b1, ipc, nj):
    hi = x @ W1[:F]          # [N, H]
    hj = x @ W1[F:]          # [N, H]
    hib = hi + b1[None, :]   # fold b1
    hj_t = np.ascontiguousarray(hj[:nj].T)    # [H, nj]
    return _layout(ipc, nj), hib, hj_t


def make_in_maps(x, W1, b1, W2, b2, W3, b3):
    import ml_dtypes
    lay, hib, hj_t = _host_prep(x, W1, b1, IPC, NJ)
    W3pad = np.zeros((H, OPAD), np.float32)
    W3pad[:, :O] = W3
    b3c = np.zeros((H,), np.float32)
    for c4 in range(4):
        b3c[32 * c4:32 * c4 + O] = b3
    in_maps = []
    for c in range(NCORES):
        cb = np.empty((H, lay["ncols_b"]), ml_dtypes.bfloat16)
        cb[:, lay["hj"]:lay["hj"] + NJ] = hj_t
        cb[:, lay["w2"]:lay["w2"] + H] = W2
        cb[:, lay["w3"]:lay["w3"] + OPAD] = W3pad
        cf = np.empty((H, lay["ncols_f"]), np.float32)
        cf[:, lay["hib"]:lay["hib"] + IPC] = hib[c * IPC:(c + 1) * IPC].T
        cf[:, lay["b2"]] = b2
        cf[:, lay["b3c"]] = b3c
        in_maps.append({"cb": cb, "cf": cf})
    return in_maps


def _unpack_y(yarr):
    """Per-core y [H, nblocks] -> [ipc] atom sums.

    Block g holds atoms (2g, 2g+1); ACC[32*(2m+h)+o, g] = sum over j-half h
    of tanh(y_o) for member m. Atom sum = sum over o<3, both halves."""
    nblocks = yarr.shape[1]
    out = np.empty(BLK * nblocks, np.float32)
    for m in range(BLK):
        rows0 = 32 * (2 * m)
        rows1 = 32 * (2 * m + 1)
        s = (yarr[rows0:rows0 + O, :].sum(axis=0)
             + yarr[rows1:rows1 + O, :].sum(axis=0))
        out[m::BLK] = s
    return out


def kernel(x, W1, b1, W2, b2, W3, b3):
    x = np.asarray(x, np.float32)
    W1 = np.asarray(W1, np.float32)
    b1 = np.asarray(b1, np.float32)
    W2 = np.asarray(W2, np.float32)
    b2 = np.asarray(b2, np.float32)
    W3 = np.asarray(W3, np.float32)
    b3 = np.asarray(b3, np.float32)

    in_maps = make_in_maps(x, W1, b1, W2, b2, W3, b3)
    nc = _get_nc(IPC, NJ)
    res = run_bass_kernel_spmd(nc, in_maps, list(range(NCORES)))
    out = np.concatenate(
        [_unpack_y(res.results[c]["y"]) for c in range(NCORES)]
    ).astype(np.float32)
    return out


# revision 10
# speedup vs baseline: 3.3777x; 1.3042x over previous
"""Trainium2 Bass kernel for nn_ConvPair (pairwise-MLP message passing).

Reference computation (N=1024 atoms, F=8 feats, H=128 hidden, O=3 out):
    hi = x @ W1[:F];  hj = x @ W1[F:]
    h  = tanh(hi[:,None,:] + hj[None,:,:] + b1)        # [N,N,H]
    h  = tanh(h @ W2 + b2)                             # [N,N,H]
    y  = tanh(h @ W3 + b3)                             # [N,N,O]
    out = y.sum(axis=(1,2))                            # [N]

Sharding: outer atom dim i split across 8 cores (128 i per core); weights
and the atom table replicated. No cross-core reduction.

Per-core pipeline, hidden-major [H=128 partitions, ...]:
  The ScalarE (ACT) is the bound engine: all ~34M tanh/core go through its
  128-lane 1 elem/cycle/lane pipe. Everything else is arranged to keep ACT
  saturated:
    z1:    DVE  tensor_scalar_add HJ + hib_i (bf16 4x mode), G=16 atom batch
    tanh1: ACT  one 16384-col instruction per batch (amortize ~352cyc fixed)
    mm2:   PE   W2 stationary bf16 (1 cyc/row + FWL), 2-i blocks into a
                [128,2048] PSUM tile (4 banks, ping-pong x2 = 8 banks)
    tanh2: ACT  tanh(psum + b2) -> h2 bf16 SBUF, one 2048-col instr per block
    mm3:   PE   pairs-on-partitions: h2 chunk [128h,128j] stationary (FWL),
                W3pad moving (N=4); 16 tiny outputs packed into the tail 64
                cols of the block's own psum tile (after tanh2 consumed them)
    tanh3: ACT  one 64-col in-place tanh per block (j on partitions now)
    red:   DVE  tensor_reduce over each atom's 32 cols -> ACC[:,i] (j-offset
                partial sums); host sums the 128 partitions per atom.
  Final [128, 128] ACC goes back via DMA.

bf16 on the matmul path is safe: tolerance is 2e-2 of max|out| ~ 999, i.e.
~20 absolute, while bf16 noise accumulates to ~0.5 here.

b1 is folded into hib host-side; b2/b3 ride the ACT bias port (b3 as a
per-partition column matching the col-tiled mm3 layout).

Wait-discipline: walrus supports limited semaphore waits on datapath
instructions; _legalize_waits hoists extras onto NoOps.
"""

import json

import numpy as np
from contextlib import ExitStack

import bass_rust
import concourse.bass as bass
import concourse.tile as tile
from concourse import mybir
from concourse.bass_utils import run_bass_kernel_spmd

f32 = mybir.dt.float32
bf16 = mybir.dt.bfloat16
Tanh = mybir.ActivationFunctionType.Tanh

N, F, H, O = 1024, 8, 128, 3
NCORES = 8
IPC = N // NCORES   # 128 atoms (i) per core
NJ = N              # full j dimension on every core
MM_N = 512          # max moving free dim per matmul (one PSUM bank)
OPAD = 4            # W3 padded 3 -> 4 cols
G = 32              # z1/tanh1 steady-state batch (atoms)
RAMP = [4, 4, 8, 16]  # first batches (cut ACT startup idle)
BLK = 2             # atoms per mm2/tanh2 block


def _layout(ipc, nj):
    """Column offsets: cb = packed bf16 block, cf = packed f32 block."""
    hj = 0
    w2 = hj + nj
    w3 = w2 + H
    ncols_b = w3 + OPAD
    hib = 0
    b2 = hib + ipc
    b3c = b2 + 1
    ncols_f = b3c + 1
    return dict(hj=hj, hib=hib, w2=w2, w3=w3, ncols_b=ncols_b,
                b2=b2, b3c=b3c, ncols_f=ncols_f)


# TPB instructions have a single 8-byte events field: 2 sync commands max
# (walrus rejects more).  Queue-engine DMA ops handle their own sync.
_MULTIWAIT_OK = {"DMACopy", "TriggeredCopy", "Call", "ISA"}


def _legalize_waits(nc):
    """Hoist excess semaphore waits from datapath instructions onto chained
    NoOps (one wait each) so every instruction fits walrus's sync budget."""
    j = json.loads(bass_rust.module_to_json_string(nc.m))
    counter = [0]

    def fix_list(insts):
        out = []
        for inst in insts:
            si = inst.get("sync_info")
            waits = (si or {}).get("on_wait", [])
            if si and len(waits) > 1 and inst.get("opcode") not in _MULTIWAIT_OK:
                for w in waits:
                    counter[0] += 1
                    out.append({
                        "debug": inst.get("debug", 0),
                        "engine": inst["engine"],
                        "ins": [],
                        "outs": [],
                        "name": f"W-hoist-{counter[0]}",
                        "opcode": "NoOp",
                        "sync_info": {"on_update": [], "on_wait": [w]},
                    })
                si["on_wait"] = []
            out.append(inst)
        return out

    def walk(o):
        if isinstance(o, dict):
            if "instructions" in o and isinstance(o["instructions"], list):
                o["instructions"] = fix_list(o["instructions"])
            for v in o.values():
                walk(v)
        elif isinstance(o, list):
            for v in o:
                walk(v)

    walk(j)
    nc.m = bass_rust.module_from_json_string(json.dumps(j))
    return counter[0]


def _build(ipc, nj, reps=1, legalize=True):
    """Build the per-core Bass program (SPMD: same program, per-core data).

    reps > 1 repeats the main i-loop (recomputing identical results); used
    only for differential timing, outputs unchanged."""
    ramp = [g for g in RAMP if sum(RAMP) <= ipc]
    sizes = list(ramp) + [G] * ((ipc - sum(ramp)) // G)
    assert sum(sizes) == ipc and all(s % BLK == 0 for s in sizes)
    assert nj == 2 * MM_N
    nblocks = ipc // BLK
    lay = _layout(ipc, nj)

    nc = bass.Bass()
    cbparam = nc.declare_dram_parameter("cb", [H, lay["ncols_b"]], bf16,
                                        isOutput=False)
    cfparam = nc.declare_dram_parameter("cf", [H, lay["ncols_f"]], f32,
                                        isOutput=False)
    yparam = nc.declare_dram_parameter("y", [H, ipc], f32, isOutput=True)

    with tile.TileContext(nc) as tc:
        with ExitStack() as ctx:
            consts = ctx.enter_context(tc.tile_pool(name="consts", bufs=1))
            z1p = ctx.enter_context(tc.tile_pool(name="z1p", bufs=2))
            h2p = ctx.enter_context(tc.tile_pool(name="h2p", bufs=3))
            scrp = ctx.enter_context(tc.tile_pool(name="scrp", bufs=1))
            accp = ctx.enter_context(tc.tile_pool(name="accp", bufs=1))
            psp = ctx.enter_context(tc.tile_pool(name="psp", bufs=3, space="PSUM"))
            ps3p = ctx.enter_context(tc.tile_pool(name="ps3p", bufs=2, space="PSUM"))

            CB = consts.tile([H, lay["ncols_b"]], bf16, tag="cb")
            CF = consts.tile([H, lay["ncols_f"]], f32, tag="cf")
            nc.sync.dma_start(out=CB, in_=cbparam[:, :])
            nc.sync.dma_start(out=CF, in_=cfparam[:, :])

            HJ = CB[:, lay["hj"]:lay["hj"] + nj]
            W2 = CB[:, lay["w2"]:lay["w2"] + H]
            W3 = CB[:, lay["w3"]:lay["w3"] + OPAD]
            B2 = CF[:, lay["b2"]:lay["b2"] + 1]
            B3C = CF[:, lay["b3c"]:lay["b3c"] + 1]

            ACC = accp.tile([H, ipc], f32)
            warm = scrp.tile([H, 1], f32, tag="warm")

            # warmup: load the tanh table (and touch the const-DMA sems on
            # ACT) outside the steady-state loop.
            nc.scalar.activation(out=warm, in_=B2, func=Tanh)

            NCH = nj // H          # 8 j-chunks per atom
            RED = 16               # atoms per ps3 tile / tanh3 group
            nbatches = len(sizes)
            starts = [sum(sizes[:b]) for b in range(nbatches)]
            for rep in range(reps):
                # z1 adds for batch 0 (ramp keeps this short)
                z1t = {}
                z1t[0] = z1p.tile([H, G, nj], bf16, name="z1", tag="z1")
                for k in range(sizes[0]):
                    nc.vector.tensor_scalar_add(
                        z1t[0][:, k, :], HJ, CF[:, lay["hib"] + k:lay["hib"] + k + 1])
                ps3 = None
                for B, gsz in enumerate(sizes):
                    z1 = z1t.pop(B)[:, :gsz, :]
                    nc.scalar.activation(out=z1, in_=z1, func=Tanh)
                    # interleave next batch's z1 adds through this batch
                    nxt = sizes[B + 1] if B + 1 < nbatches else 0
                    if nxt:
                        z1t[B + 1] = z1p.tile([H, G, nj], bf16, name="z1", tag="z1")
                    adds_done = 0
                    for k in range(gsz):
                        t = starts[B] + k
                        s = t % RED
                        if s == 0:
                            ps3 = ps3p.tile([H, RED * NCH * OPAD], f32)
                        P = psp.tile([H, nj], f32)
                        for h in range(2):
                            nc.tensor.matmul(
                                P[:, h * MM_N:(h + 1) * MM_N],
                                W2, z1[:, k, h * MM_N:(h + 1) * MM_N],
                                start=True, stop=True)
                        h2 = h2p.tile([H, nj], bf16)
                        nc.scalar.activation(out=h2, in_=P, func=Tanh, bias=B2)
                        for c in range(NCH):
                            off = (s * NCH + c) * OPAD
                            nc.tensor.matmul(
                                ps3[:, off:off + OPAD],
                                h2[:, c * H:(c + 1) * H], W3,
                                start=True, stop=True)
                        # spread next batch's z1 adds across this batch
                        want = (nxt * (k + 1)) // gsz
                        for a in range(adds_done, want):
                            i = starts[B + 1] + a
                            nc.vector.tensor_scalar_add(
                                z1t[B + 1][:, a, :], HJ,
                                CF[:, lay["hib"] + i:lay["hib"] + i + 1])
                        adds_done = want
                        if s == RED - 1:
                            nc.scalar.activation(out=ps3, in_=ps3, func=Tanh)
                            for q in range(RED):
                                tq = t - (RED - 1) + q
                                nc.vector.tensor_reduce(
                                    out=ACC[:, tq:tq + 1],
                                    in_=ps3[:, q * NCH * OPAD:(q + 1) * NCH * OPAD],
                                    axis=mybir.AxisListType.X,
                                    op=mybir.AluOpType.add)

            nc.sync.dma_start(out=yparam[:, :], in_=ACC)

    if legalize:
        _legalize_waits(nc)
    return nc


_NC_CACHE = {}


def _get_nc(ipc, nj):
    key = (ipc, nj)
    if key not in _NC_CACHE:
        _NC_CACHE[key] = _build(ipc, nj)
    return _NC_CACHE[key]


def _host_prep(x, W1, b1, ipc, nj):
    hi = x @ W1[:F]          # [N, H]
    hj = x @ W1[F:]          # [N, H]
    hib = hi + b1[None, :]   # fold b1
    hj_t = np.ascontiguousarray(hj[:nj].T)    # [H, nj]
    return _layout(ipc, nj), hib, hj_t


def make_in_maps(x, W1, b1, W2, b2, W3, b3):
    import ml_dtypes
    lay, hib, hj_t = _host_prep(x, W1, b1, IPC, NJ)
    W3pad = np.zeros((H, OPAD), np.float32)
    W3pad[:, :O] = W3
    in_maps = []
    for c in range(NCORES):
        cb = np.empty((H, lay["ncols_b"]), ml_dtypes.bfloat16)
        cb[:, lay["hj"]:lay["hj"] + NJ] = hj_t
        cb[:, lay["w2"]:lay["w2"] + H] = W2
        cb[:, lay["w3"]:lay["w3"] + OPAD] = W3pad
        cf = np.empty((H, lay["ncols_f"]), np.float32)
        cf[:, lay["hib"]:lay["hib"] + IPC] = hib[c * IPC:(c + 1) * IPC].T
        cf[:, lay["b2"]] = b2
        cf[:, lay["b3c"]] = 0.0
        in_maps.append({"cb": cb, "cf": cf})
    return in_maps


def _unpack_y(yarr):
    """Per-core y [H, ipc] -> [ipc]: ACC[p, i] = sum over (chunk, o) of
    tanh(y[i, c*128+p, o]); atom sum = sum over the 128 partitions."""
    return yarr.sum(axis=0).astype(np.float32)


def kernel(x, W1, b1, W2, b2, W3, b3):
    x = np.asarray(x, np.float32)
    W1 = np.asarray(W1, np.float32)
    b1 = np.asarray(b1, np.float32)
    W2 = np.asarray(W2, np.float32)
    b2 = np.asarray(b2, np.float32)
    W3 = np.asarray(W3, np.float32)
    b3 = np.asarray(b3, np.float32)

    if np.any(b3 != 0.0):
        # Never hit for this problem (spec fills b3 with zeros); exact
        # numpy fallback keeps the kernel correct for arbitrary inputs.
        return _numpy_ref(x, W1, b1, W2, b2, W3, b3)

    in_maps = make_in_maps(x, W1, b1, W2, b2, W3, b3)
    nc = _get_nc(IPC, NJ)
    res = run_bass_kernel_spmd(nc, in_maps, list(range(NCORES)))
    out = np.concatenate(
        [_unpack_y(res.results[c]["y"]) for c in range(NCORES)]
    ).astype(np.float32)
    return out


def _numpy_ref(x, W1, b1, W2, b2, W3, b3):
    hi = x @ W1[:F]
    hj = x @ W1[F:]
    out = np.empty((N,), np.float32)
    for i in range(N):
        h = np.tanh(hi[i][None, :] + hj + b1[None, :])
        h = np.tanh(h @ W2 + b2[None, :])
        y = np.tanh(h @ W3 + b3[None, :])
        out[i] = y.sum()
    return out
